# revision 1
# baseline (speedup 1.0000x reference)
"""AttentionPooling (segment softmax + weighted segment-sum) on 8 trn2 cores.

Strategy: shard nodes across cores at segment-aligned cuts (batch is sorted),
single pass over x per core. Per 128-node tile: PE transpose -> MLP scores ->
exp -> scatter-matmul (A_e^T @ x) accumulating [128seg, 256+1] in PSUM over a
124-tile window; windows chain via a carried partial row for the straddling
segment. Normalization (U/Z) on device. No collectives; host scatters the
per-window rows into the final [4096, 256] output.
"""

import numpy as np

# ---------------------------------------------------------------- constants
N_FULL = 1_000_000
D = 256
H = 128
G = 4096
NCORES = 8
P = 128

TILES = 992                 # node tiles per core
NC_PAD = TILES * P          # 126976 padded nodes per core
WINDOWS = 8
TPW = TILES // WINDOWS      # 124 tiles per window
WIN_NODES = TPW * P         # 15872
SUPER = 31                  # node tiles per DMA super-tile
SUPERS_PER_WIN = TPW // SUPER   # 4
OUT_ROWS = WINDOWS * P      # 1024 rows per core
EPS = 1e-30


def _set_config(tiles, windows, super_):
    """Reconfigure problem tiling (testing only; defaults are production)."""
    global TILES, NC_PAD, WINDOWS, TPW, WIN_NODES, SUPER, SUPERS_PER_WIN, OUT_ROWS
    TILES, WINDOWS, SUPER = tiles, windows, super_
    NC_PAD = TILES * P
    TPW = TILES // WINDOWS
    assert TPW * WINDOWS == TILES
    WIN_NODES = TPW * P
    SUPERS_PER_WIN = TPW // super_
    assert SUPERS_PER_WIN * super_ == TPW
    OUT_ROWS = WINDOWS * P
    _NC_CACHE.clear()


# ---------------------------------------------------------------- host plan
def _plan(batch):
    """batch: sorted int array [N]. Returns per-core planning dicts."""
    batch = np.asarray(batch).astype(np.int64).ravel()
    n = batch.shape[0]
    # all segment-start positions (including 0 and n)
    change = np.flatnonzero(np.diff(batch)) + 1
    bounds = np.concatenate([[0], change, [n]])
    cuts = [0]
    for c in range(1, NCORES):
        tgt = c * n // NCORES
        i = np.searchsorted(bounds, tgt)
        lo = bounds[i - 1] if i > 0 else bounds[0]
        hi = bounds[min(i, len(bounds) - 1)]
        cut = int(lo if (tgt - lo) <= (hi - tgt) else hi)
        cuts.append(cut)
    cuts.append(n)
    for i in range(NCORES):
        assert cuts[i] < cuts[i + 1], f"empty core shard {i}: {cuts}"
        assert cuts[i + 1] - cuts[i] <= NC_PAD, (
            f"core {i} shard {cuts[i + 1] - cuts[i]} > NC_PAD={NC_PAD}"
        )

    plans = []
    for c in range(NCORES):
        lo, hi = cuts[c], cuts[c + 1]
        n_c = hi - lo
        local = batch[lo:hi]
        rel = np.full(NC_PAD, -1.0, np.float32)
        bases = np.full(WINDOWS, -1, np.int64)
        for w in range(WINDOWS):
            a = w * WIN_NODES
            b = min((w + 1) * WIN_NODES, n_c)
            if a >= n_c:
                continue
            base = int(local[a])
            bases[w] = base
            r = local[a:b] - base
            assert r.min() >= 0 and r.max() < P, (
                f"core {c} window {w}: {P} seg rows exceeded (max rel {r.max()})"
            )
            rel[a:b] = r.astype(np.float32)

        last_seg = int(local[-1])
        onehot = np.zeros((P, WINDOWS), np.float32)
        valid = []  # (global_seg_start, nrows) per window
        for w in range(WINDOWS):
            if bases[w] < 0:
                valid.append((0, 0))
                continue
            nxt = bases[w + 1] if (w + 1 < WINDOWS and bases[w + 1] >= 0) else -1
            if nxt >= 0:
                diff = int(nxt - bases[w])
                assert 0 < diff < P, f"core {c} window {w}: carry diff {diff}"
                onehot[diff, w] = 1.0
                hi_seg = nxt
            else:
                hi_seg = last_seg + 1
            nrows = hi_seg - int(bases[w])
            assert 0 < nrows <= P
            valid.append((int(bases[w]), int(nrows)))

        # rel_seg rearranged so partition p, col t = rel[t*P + p]
        rel_arr = rel.reshape(TILES, P).T.copy()  # [P, TILES]
        plans.append(
            dict(lo=lo, hi=hi, n_c=n_c, rel_arr=rel_arr, onehot=onehot, valid=valid)
        )
    return plans


def _make_in_maps(x, W1, b1, W2, b2, plans):
    x = np.ascontiguousarray(np.asarray(x), dtype=np.float32)
    W1 = np.ascontiguousarray(np.asarray(W1), dtype=np.float32)
    b1 = np.ascontiguousarray(np.asarray(b1), dtype=np.float32).reshape(H, 1)
    W2 = np.ascontiguousarray(np.asarray(W2), dtype=np.float32).reshape(H, 1)
    W2 = np.repeat(W2, 2, axis=1)
    b2 = np.ascontiguousarray(np.asarray(b2), dtype=np.float32).reshape(1, 1)
    in_maps = []
    for pl in plans:
        xp = np.zeros((NC_PAD, D), np.float32)
        xp[: pl["n_c"]] = x[pl["lo"] : pl["hi"]]
        in_maps.append(
            {
                "x": xp,
                "relseg": pl["rel_arr"],
                "onehot": pl["onehot"],
                "w1": W1,
                "b1": b1,
                "w2": W2,
                "b2": b2,
            }
        )
    return in_maps


def _assemble(outs, plans, dtype):
    final = np.zeros((G, D), dtype)
    for pl, o in zip(plans, outs):
        for w, (g0, nrows) in enumerate(pl["valid"]):
            if nrows:
                final[g0 : g0 + nrows] = o[w * P : w * P + nrows]
    return final


# ------------------------------------------------------------ numpy emulator
def _emulate(inputs):
    """Pure-numpy emulation of the device program (for logic validation)."""
    x = np.asarray(inputs["x"], np.float32)
    W1 = np.asarray(inputs["W1"], np.float32)
    b1 = np.asarray(inputs["b1"], np.float32)
    W2 = np.asarray(inputs["W2"], np.float32)
    b2 = np.asarray(inputs["b2"], np.float32)
    plans = _plan(inputs["batch"])
    in_maps = _make_in_maps(x, W1, b1, W2, b2, plans)
    outs = []
    cols = np.arange(P, dtype=np.float32)[None, :]
    for im in in_maps:
        xp = im["x"]
        rel = im["relseg"].T.reshape(-1)  # [NC_PAD] node order
        h = np.tanh(xp @ W1 + b1[None, :].reshape(1, H))
        s = (h @ W2).ravel() + float(b2.ravel()[0])
        e = np.exp(s)
        out = np.zeros((OUT_ROWS, D), np.float32)
        carry = np.zeros(D + 1, np.float32)
        for w in range(WINDOWS):
            uz = np.zeros((P, D + 1), np.float32)
            a, b = w * WIN_NODES, (w + 1) * WIN_NODES
            A = (cols == rel[a:b, None]).astype(np.float32) * e[a:b, None]
            uz[:, :D] = A.T @ xp[a:b]
            uz[:, D] = A.sum(axis=0)
            uz[0] += carry
            carry = im["onehot"][:, w] @ uz
            out[w * P : (w + 1) * P] = uz[:, :D] / (uz[:, D : D + 1] + EPS)
        outs.append(out)
    return _assemble(outs, plans, np.float32)


# ------------------------------------------------------------- bass program
_NC_CACHE = {}

AE_ENGINE = "vector"  # engine for the A_e scatter-build ("gpsimd" or "vector")


def _build_nc():
    if "nc" in _NC_CACHE:
        return _NC_CACHE["nc"]
    import concourse.bacc as bacc
    import concourse.mybir as mybir
    import concourse.tile as tile
    from concourse.masks import make_identity

    f32 = mybir.dt.float32
    f32r = mybir.dt.float32r
    AF = mybir.ActivationFunctionType
    ALU = mybir.AluOpType

    assert TPW % 4 == 0 and TPW % SUPER == 0

    nc = bacc.Bacc(None, target_bir_lowering=False)

    x_d = nc.dram_tensor("x", [NC_PAD, D], f32r, kind="ExternalInput")
    rel_d = nc.dram_tensor("relseg", [P, TILES], f32, kind="ExternalInput")
    oh_d = nc.dram_tensor("onehot", [P, WINDOWS], f32, kind="ExternalInput")
    w1_d = nc.dram_tensor("w1", [D, H], f32r, kind="ExternalInput")
    b1_d = nc.dram_tensor("b1", [H, 1], f32, kind="ExternalInput")
    w2_d = nc.dram_tensor("w2", [H, 2], f32r, kind="ExternalInput")
    b2_d = nc.dram_tensor("b2", [1, 1], f32, kind="ExternalInput")
    out_d = nc.dram_tensor("out", [OUT_ROWS, D], f32, kind="ExternalOutput")

    with tile.TileContext(nc) as tc:
        with (
            tc.tile_pool(name="singles", bufs=1) as singles,
            tc.tile_pool(name="xsup", bufs=3) as xpool,
            tc.tile_pool(name="xt_sb", bufs=2) as xt_pool,
            tc.tile_pool(name="hb", bufs=2) as hb_pool,
            tc.tile_pool(name="e", bufs=2) as e_pool,
            tc.tile_pool(name="ae", bufs=4) as ae_pool,
            tc.tile_pool(name="flush", bufs=2) as flush_pool,
            tc.tile_pool(name="ps_xt", bufs=2, space="PSUM") as ps_xt,
            tc.tile_pool(name="ps_h", bufs=2, space="PSUM") as ps_h,
            tc.tile_pool(name="ps_s", bufs=1, space="PSUM") as ps_s,
            tc.tile_pool(name="ps_uz", bufs=2, space="PSUM") as ps_uz,
            tc.tile_pool(name="ps_c", bufs=1, space="PSUM") as ps_c,
        ):
            ident_f = singles.tile([P, P], f32)
            make_identity(nc, ident_f[:])
            ident = singles.tile([P, P], f32r)
            nc.vector.tensor_copy(out=ident[:], in_=ident_f[:])
            iota_i = singles.tile([P, P], mybir.dt.int32)
            nc.gpsimd.iota(iota_i[:], pattern=[[1, P]], base=0, channel_multiplier=0)
            iota_f = singles.tile([P, P], f32)
            nc.vector.tensor_copy(out=iota_f[:], in_=iota_i[:])

            w1_sb = singles.tile([P, 2, H], f32r)
            w1_r = w1_d[:].rearrange("(c k) m -> c k m", c=2)
            nc.sync.dma_start(out=w1_sb[:, 0, :], in_=w1_r[0])
            nc.sync.dma_start(out=w1_sb[:, 1, :], in_=w1_r[1])
            b1_sb = singles.tile([P, 1], f32)
            nc.sync.dma_start(out=b1_sb[:], in_=b1_d[:])
            w2_sb = singles.tile([P, 2], f32r)
            nc.sync.dma_start(out=w2_sb[:], in_=w2_d[:])
            b2_sb = singles.tile([P, 1], f32)
            nc.sync.dma_start(out=b2_sb[:], in_=b2_d[:].to_broadcast([P, 1]))
            oh_sb = singles.tile([P, WINDOWS], f32)
            nc.sync.dma_start(out=oh_sb[:], in_=oh_d[:])
            rel_sb = singles.tile([P, TILES], f32)
            nc.sync.dma_start(out=rel_sb[:], in_=rel_d[:])
            ones_sb = singles.tile([P, 1], f32)
            nc.vector.memset(ones_sb[:], 1.0)
            carry_sb = singles.tile([1, D + 1], f32)
            nc.vector.memset(carry_sb[:], 0.0)

            x_r = x_d[:].rearrange("(s t p) c -> s p t c", p=P, t=SUPER)
            ae_eng = nc.gpsimd if AE_ENGINE == "gpsimd" else nc.vector

            sup_cache = {}

            def get_sup(sg):
                if sg not in sup_cache:
                    t = xpool.tile([P, SUPER, D + 2], f32r)
                    nc.sync.dma_start(out=t[:, :, 0:D], in_=x_r[sg])
                    nc.gpsimd.tensor_copy(
                        out=t[:, :, D : D + 2],
                        in_=ones_sb[:].to_broadcast([P, SUPER, 2]),
                    )
                    sup_cache[sg] = t
                return sup_cache[sg]

            for w in range(WINDOWS):
                uz_ps = ps_uz.tile([P, D + 2], f32)
                for g in range(TPW // 4):
                    xt_sb = xt_pool.tile([P, 2, 4, P], f32r)
                    for pair in range(2):
                        xt_ps = ps_xt.tile([P, 2, 2, P], f32r)
                        for t2 in range(2):
                            gt = w * TPW + g * 4 + pair * 2 + t2
                            sup = get_sup(gt // SUPER)
                            slot = gt % SUPER
                            for k in range(2):
                                nc.tensor.transpose(
                                    out=xt_ps[:, k, t2, :],
                                    in_=sup[:, slot, k * P : (k + 1) * P],
                                    identity=ident[:],
                                )
                        p2 = pair * 2
                        nc.scalar.activation(
                            out=xt_sb[:, 0, p2 : p2 + 2, :],
                            in_=xt_ps[:, 0, :, :],
                            func=AF.Copy,
                        )
                        nc.vector.tensor_copy(
                            out=xt_sb[:, 1, p2 : p2 + 2, :], in_=xt_ps[:, 1, :, :]
                        )
                    # ---- h = tanh(x @ W1 + b1) for 4 tiles, layout [hid, 4*nodes]
                    h_ps = ps_h.tile([P, 4, P], f32)
                    for k in range(2):
                        nc.tensor.matmul(
                            out=h_ps[:],
                            lhsT=w1_sb[:, k, :],
                            rhs=xt_sb[:, k, :, :],
                            start=(k == 0),
                            stop=(k == 1),
                        )
                    hb = hb_pool.tile([P, 4, P], f32r)
                    nc.scalar.activation(
                        out=hb[:], in_=h_ps[:], func=AF.Tanh, bias=b1_sb[:], scale=1.0
                    )
                    # ---- s per tile -> [nodes, 2]; e = exp(s + b2) batched
                    s_ps = ps_s.tile([P, 4, 2], f32)
                    for t in range(4):
                        nc.tensor.matmul(
                            out=s_ps[:, t, :],
                            lhsT=hb[:, t, :],
                            rhs=w2_sb[:],
                            start=True,
                            stop=True,
                        )
                    e_sb = e_pool.tile([P, 4], f32)
                    nc.scalar.activation(
                        out=e_sb[:],
                        in_=s_ps[:, :, 0],
                        func=AF.Exp,
                        bias=b2_sb[:],
                        scale=1.0,
                    )
                    # ---- A_e + pooling matmul per tile
                    for t in range(4):
                        ti = g * 4 + t
                        gt = w * TPW + ti
                        ae = ae_pool.tile([P, P], f32r)
                        ae_eng.tensor_scalar(
                            out=ae[:],
                            in0=iota_f[:],
                            scalar1=rel_sb[:, gt : gt + 1],
                            scalar2=e_sb[:, t : t + 1],
                            op0=ALU.is_equal,
                            op1=ALU.mult,
                        )
                        sup = get_sup(gt // SUPER)
                        nc.tensor.matmul(
                            out=uz_ps[:],
                            lhsT=ae[:],
                            rhs=sup[:, gt % SUPER, :],
                            start=(ti == 0),
                            stop=(ti == TPW - 1),
                        )
                # ---- flush window w
                uz_sb = flush_pool.tile([P, D + 1], f32)
                nc.vector.tensor_copy(out=uz_sb[:], in_=uz_ps[:, 0 : D + 1])
                nc.vector.tensor_add(
                    out=uz_sb[0:1, :], in0=uz_sb[0:1, :], in1=carry_sb[:]
                )
                c_ps = ps_c.tile([1, D + 1], f32)
                nc.tensor.matmul(
                    out=c_ps[:],
                    lhsT=oh_sb[:, w : w + 1],
                    rhs=uz_sb[:],
                    start=True,
                    stop=True,
                )
                nc.vector.tensor_copy(out=carry_sb[:], in_=c_ps[:])
                recip = flush_pool.tile([P, 1], f32)
                nc.vector.tensor_scalar_add(
                    out=recip[:], in0=uz_sb[:, D : D + 1], scalar1=EPS
                )
                nc.vector.reciprocal(out=recip[:], in_=recip[:])
                outw = flush_pool.tile([P, D], f32)
                nc.vector.tensor_scalar_mul(
                    out=outw[:], in0=uz_sb[:, 0:D], scalar1=recip[:]
                )
                nc.sync.dma_start(out=out_d[w * P : (w + 1) * P, :], in_=outw[:])

    nc.finalize()
    _NC_CACHE["nc"] = nc
    return nc


def _run(inputs, trace=False):
    from concourse.bass_utils import run_bass_kernel_spmd

    x = inputs["x"]
    plans = _plan(inputs["batch"])
    in_maps = _make_in_maps(
        x, inputs["W1"], inputs["b1"], inputs["W2"], inputs["b2"], plans
    )
    nc = _build_nc()
    res = run_bass_kernel_spmd(
        nc, in_maps, core_ids=list(range(NCORES)), trace=trace
    )
    outs = [r["out"] for r in res.results]
    final = _assemble(outs, plans, np.float32)
    return final, res


def kernel(**inputs):
    return _run(inputs, trace=False)[0]



# revision 2
# speedup vs baseline: 1.4809x; 1.4809x over previous
"""AttentionPooling (segment softmax + weighted segment-sum) on 8 trn2 cores.

Strategy: shard nodes equally across cores (no segment alignment needed).
Host pre-builds TWO bf16 layouts of x: natural [node, D+2ones] in super-tile
order (scatter rhs) and transposed [D, node] in chunk order (score-MLP rhs).
Device per 128-node tile: h = tanh(W1^T x^T) -> s = hb^T w2 -> e = exp(s) ->
A_e one-hot scatter matmul accumulating raw [128seg, D+2] U/Z in PSUM per
124-tile window. Windows dump raw U/Z; host divides U/Z and scatter-adds
window rows into the final [4096, 256] output (straddling segments merge by
addition). No collectives.
"""

import numpy as np
import ml_dtypes

BF16 = ml_dtypes.bfloat16

# ---------------------------------------------------------------- constants
N_FULL = 1_000_000
D = 256
H = 128
G = 4096
NCORES = 8
P = 128

NC_NODES = N_FULL // NCORES  # 125000 real nodes per core
TILES = 992                  # node tiles per core
NC_PAD = TILES * P           # 126976 padded nodes per core
WINDOWS = 8
TPW = TILES // WINDOWS       # 124 tiles per window
WIN_NODES = TPW * P          # 15872
SUPER = 31                   # xn tiles per DMA super-tile
CHUNK = 32                   # xt tiles per DMA chunk
N_SUPERS = TILES // SUPER    # 32
N_CHUNKS = TILES // CHUNK    # 31
GROUP = 4                    # tiles per h-matmul group
CA = D + 2                   # augmented cols (x | 1 | 1)
OUT_ROWS = WINDOWS * P       # 1024 rows per core
EPS = 1e-30


# ---------------------------------------------------------------- host plan
def _plan(batch):
    """batch: sorted int array [N]. Per-core window bases/rows + rel map."""
    batch = np.asarray(batch).astype(np.int64).ravel()
    n = batch.shape[0]
    assert n == N_FULL
    plans = []
    for c in range(NCORES):
        lo = c * NC_NODES
        hi = lo + NC_NODES
        local = batch[lo:hi]
        rel = np.full(NC_PAD, -1.0, np.float32)
        bases = np.zeros(WINDOWS, np.int64)
        nrows = np.zeros(WINDOWS, np.int64)
        for w in range(WINDOWS):
            a = w * WIN_NODES
            b = min((w + 1) * WIN_NODES, NC_NODES)
            assert a < NC_NODES
            base = int(local[a])
            r = local[a:b] - base
            assert r.min() >= 0 and r.max() < P, (
                f"core {c} window {w}: {P} seg rows exceeded (max rel {r.max()})"
            )
            rel[a:b] = r.astype(np.float32)
            bases[w] = base
            nrows[w] = int(local[b - 1]) - base + 1
        rel_arr = rel.reshape(TILES, P).T.copy()  # [P, TILES]
        plans.append(dict(lo=lo, hi=hi, rel_arr=rel_arr, bases=bases, nrows=nrows))
    return plans


def _make_in_maps(x, W1, b1, W2, b2, plans):
    x = np.ascontiguousarray(np.asarray(x), dtype=np.float32)
    W1 = np.asarray(W1, dtype=np.float32).astype(BF16)          # [D, H]
    b1 = np.asarray(b1, dtype=np.float32).reshape(H, 1)
    W2 = np.asarray(W2, dtype=np.float32).reshape(H, 1)
    W2 = np.repeat(W2, 2, axis=1).astype(BF16)                  # [H, 2]
    b2 = np.asarray(b2, dtype=np.float32).reshape(1, 1)
    in_maps = []
    for pl in plans:
        xs = np.zeros((NC_PAD, D), np.float32)
        xs[:NC_NODES] = x[pl["lo"] : pl["hi"]]
        # natural augmented layout, super-tile order:
        # xn[s*128 + p, t*258 + c] = xaug[s*3968 + t*128 + p, c]
        xa = np.ones((NC_PAD, CA), np.float32)
        xa[:, :D] = xs
        xa[NC_NODES:] = 0.0
        xn = (
            xa.reshape(N_SUPERS, SUPER, P, CA)
            .transpose(0, 2, 1, 3)
            .reshape(N_SUPERS * P, SUPER * CA)
            .astype(BF16)
        )
        # transposed layout, chunk order:
        # xt[ch*128 + d, k*4096 + j] = xs[ch*4096 + j, k*128 + d]
        xt = (
            xs.reshape(N_CHUNKS, CHUNK * P, 2, P)
            .transpose(0, 3, 2, 1)
            .reshape(N_CHUNKS * P, 2 * CHUNK * P)
            .astype(BF16)
        )
        in_maps.append(
            {
                "xn": xn,
                "xt": xt,
                "relseg": pl["rel_arr"],
                "w1": W1,
                "b1": b1,
                "w2": W2,
                "b2": b2,
            }
        )
    return in_maps


def _assemble(outs, plans, dtype):
    U = np.zeros((G, D), np.float64)
    Z = np.zeros((G,), np.float64)
    for pl, o in zip(plans, outs):
        o = np.asarray(o, np.float64)
        for w in range(WINDOWS):
            g0 = int(pl["bases"][w])
            nr = int(pl["nrows"][w])
            rows = o[w * P : w * P + nr]
            U[g0 : g0 + nr] += rows[:, :D]
            Z[g0 : g0 + nr] += rows[:, D]
    y = U / (Z[:, None] + EPS)
    return y.astype(dtype)


# ------------------------------------------------------------ numpy emulator
def _emulate(inputs):
    """Pure-numpy emulation of the device program (for logic validation)."""
    W1 = np.asarray(inputs["W1"], np.float32)
    b1 = np.asarray(inputs["b1"], np.float32)
    b2 = np.asarray(inputs["b2"], np.float32)
    plans = _plan(inputs["batch"])
    in_maps = _make_in_maps(
        inputs["x"], W1, b1, inputs["W2"], b2, plans
    )
    outs = []
    cols = np.arange(P, dtype=np.float32)[None, :]
    for im in in_maps:
        # reconstruct device views from the DMA layouts
        xn = (
            np.asarray(im["xn"], np.float32)
            .reshape(N_SUPERS, P, SUPER, CA)
            .transpose(0, 2, 1, 3)
            .reshape(NC_PAD, CA)
        )
        xt = (
            np.asarray(im["xt"], np.float32)
            .reshape(N_CHUNKS, P, 2, CHUNK * P)
            .transpose(0, 3, 2, 1)
            .reshape(NC_PAD, D)
        )
        w1 = np.asarray(im["w1"], np.float32)
        w2 = np.asarray(im["w2"], np.float32)[:, 0]
        rel = im["relseg"].T.reshape(-1)  # [NC_PAD] node order
        h = np.tanh(
            (xt.astype(BF16).astype(np.float32) @ w1) + b1.reshape(1, H)
        ).astype(BF16).astype(np.float32)
        s = h @ w2 + float(b2.ravel()[0])
        e = np.exp(s).astype(np.float32)
        out = np.zeros((OUT_ROWS, CA), np.float32)
        for w in range(WINDOWS):
            a, b = w * WIN_NODES, (w + 1) * WIN_NODES
            A = (cols == rel[a:b, None]).astype(np.float32) * e[a:b, None]
            A = A.astype(BF16).astype(np.float32)
            out[w * P : (w + 1) * P] = A.T @ xn[a:b]
        outs.append(out)
    return _assemble(outs, plans, np.float32)


# ------------------------------------------------------------- bass program
_NC_CACHE = {}


def _build_nc():
    if "nc" in _NC_CACHE:
        return _NC_CACHE["nc"]
    import concourse.bacc as bacc
    import concourse.mybir as mybir
    import concourse.tile as tile

    f32 = mybir.dt.float32
    bf16 = mybir.dt.bfloat16
    AF = mybir.ActivationFunctionType
    ALU = mybir.AluOpType

    nc = bacc.Bacc(None, target_bir_lowering=False)

    xn_d = nc.dram_tensor("xn", [N_SUPERS * P, SUPER * CA], bf16, kind="ExternalInput")
    xt_d = nc.dram_tensor("xt", [N_CHUNKS * P, 2 * CHUNK * P], bf16, kind="ExternalInput")
    rel_d = nc.dram_tensor("relseg", [P, TILES], f32, kind="ExternalInput")
    w1_d = nc.dram_tensor("w1", [D, H], bf16, kind="ExternalInput")
    b1_d = nc.dram_tensor("b1", [H, 1], f32, kind="ExternalInput")
    w2_d = nc.dram_tensor("w2", [H, 2], bf16, kind="ExternalInput")
    b2_d = nc.dram_tensor("b2", [1, 1], f32, kind="ExternalInput")
    out_d = nc.dram_tensor("out", [OUT_ROWS, CA], f32, kind="ExternalOutput")

    with tile.TileContext(nc) as tc:
        with (
            tc.tile_pool(name="singles", bufs=1) as singles,
            tc.tile_pool(name="xn_sup", bufs=3) as xn_pool,
            tc.tile_pool(name="xt_chk", bufs=3) as xt_pool,
            tc.tile_pool(name="hb", bufs=3) as hb_pool,
            tc.tile_pool(name="e", bufs=3) as e_pool,
            tc.tile_pool(name="ae", bufs=6) as ae_pool,
            tc.tile_pool(name="flush", bufs=2) as flush_pool,
            tc.tile_pool(name="ps_h", bufs=2, space="PSUM") as ps_h,
            tc.tile_pool(name="ps_s", bufs=2, space="PSUM") as ps_s,
            tc.tile_pool(name="ps_uz", bufs=2, space="PSUM") as ps_uz,
        ):
            iota_i = singles.tile([P, P], mybir.dt.int32)
            nc.gpsimd.iota(iota_i[:], pattern=[[1, P]], base=0, channel_multiplier=0)
            iota_bf = singles.tile([P, P], bf16)
            nc.vector.tensor_copy(out=iota_bf[:], in_=iota_i[:])

            w1_sb = singles.tile([P, 2, H], bf16)
            w1_r = w1_d[:].rearrange("(k d) m -> k d m", k=2)
            nc.sync.dma_start(out=w1_sb[:, 0, :], in_=w1_r[0])
            nc.sync.dma_start(out=w1_sb[:, 1, :], in_=w1_r[1])
            b1_sb = singles.tile([P, 1], f32)
            nc.sync.dma_start(out=b1_sb[:], in_=b1_d[:])
            w2_sb = singles.tile([P, 2], bf16)
            nc.sync.dma_start(out=w2_sb[:], in_=w2_d[:])
            b2_sb = singles.tile([P, 1], f32)
            nc.sync.dma_start(out=b2_sb[:], in_=b2_d[:].to_broadcast([P, 1]))
            rel_sb = singles.tile([P, TILES], f32)
            nc.sync.dma_start(out=rel_sb[:], in_=rel_d[:])

            sup_cache = {}
            chk_cache = {}

            def get_sup(sg):
                if sg not in sup_cache:
                    t = xn_pool.tile([P, SUPER, CA], bf16)
                    nc.sync.dma_start(
                        out=t[:],
                        in_=xn_d[sg * P : (sg + 1) * P, :].rearrange(
                            "p (t c) -> p t c", t=SUPER
                        ),
                    )
                    sup_cache[sg] = t
                return sup_cache[sg]

            def get_chk(cg):
                if cg not in chk_cache:
                    t = xt_pool.tile([P, 2, CHUNK * P], bf16)
                    nc.sync.dma_start(
                        out=t[:],
                        in_=xt_d[cg * P : (cg + 1) * P, :].rearrange(
                            "p (k j) -> p k j", k=2
                        ),
                    )
                    chk_cache[cg] = t
                return chk_cache[cg]

            for w in range(WINDOWS):
                uz_ps = ps_uz.tile([P, CA], f32)
                for g in range(TPW // GROUP):
                    g_abs = w * (TPW // GROUP) + g
                    t0 = g_abs * GROUP  # first tile of group
                    chk = get_chk(t0 // CHUNK)
                    off = (t0 % CHUNK) * P
                    # ---- h = tanh(x @ W1 + b1): [hid, 4*nodes]
                    h_ps = ps_h.tile([P, GROUP, P], f32)
                    for k in range(2):
                        nc.tensor.matmul(
                            out=h_ps[:],
                            lhsT=w1_sb[:, k, :],
                            rhs=chk[:, k, off : off + GROUP * P],
                            start=(k == 0),
                            stop=(k == 1),
                        )
                    hb = hb_pool.tile([P, GROUP, P], bf16)
                    nc.scalar.activation(
                        out=hb[:], in_=h_ps[:], func=AF.Tanh, bias=b1_sb[:], scale=1.0
                    )
                    # ---- s per tile -> [nodes, 2]; e = exp(s + b2)
                    s_ps = ps_s.tile([P, GROUP, 2], f32)
                    for t in range(GROUP):
                        nc.tensor.matmul(
                            out=s_ps[:, t, :],
                            lhsT=hb[:, t, :],
                            rhs=w2_sb[:],
                            start=True,
                            stop=True,
                        )
                    e_sb = e_pool.tile([P, GROUP], f32)
                    nc.scalar.activation(
                        out=e_sb[:],
                        in_=s_ps[:, :, 0],
                        func=AF.Exp,
                        bias=b2_sb[:],
                        scale=1.0,
                    )
                    # ---- A_e build + scatter matmul per tile
                    for t in range(GROUP):
                        gt = t0 + t
                        ae = ae_pool.tile([P, P], bf16)
                        nc.vector.tensor_scalar(
                            out=ae[:],
                            in0=iota_bf[:],
                            scalar1=rel_sb[:, gt : gt + 1],
                            scalar2=e_sb[:, t : t + 1],
                            op0=ALU.is_equal,
                            op1=ALU.mult,
                        )
                        sup = get_sup(gt // SUPER)
                        nc.tensor.matmul(
                            out=uz_ps[:],
                            lhsT=ae[:],
                            rhs=sup[:, gt % SUPER, :],
                            start=(gt % TPW == 0),
                            stop=(gt % TPW == TPW - 1),
                        )
                # ---- dump raw window U/Z
                uz_sb = flush_pool.tile([P, CA], f32)
                nc.vector.tensor_copy(out=uz_sb[:], in_=uz_ps[:])
                nc.sync.dma_start(out=out_d[w * P : (w + 1) * P, :], in_=uz_sb[:])

    nc.finalize()
    _NC_CACHE["nc"] = nc
    return nc


def _run(inputs, trace=False):
    from concourse.bass_utils import run_bass_kernel_spmd

    plans = _plan(inputs["batch"])
    in_maps = _make_in_maps(
        inputs["x"], inputs["W1"], inputs["b1"], inputs["W2"], inputs["b2"], plans
    )
    nc = _build_nc()
    res = run_bass_kernel_spmd(
        nc, in_maps, core_ids=list(range(NCORES)), trace=trace
    )
    outs = [r["out"] for r in res.results]
    final = _assemble(outs, plans, np.float32)
    return final, res


def kernel(**inputs):
    return _run(inputs, trace=False)[0]
